# revision 29
# baseline (speedup 1.0000x reference)
"""Trainium2 Bass kernel: DeepSeek-style MoE layer (16 experts, top-2).

Strategy (expert-parallel, 8 cores):
  - Host computes the router (softmax + top-2 + renorm) in fp32 numpy and
    builds the token dispatch.  Experts are paired large-with-small onto
    cores; slot A holds up to 1152 tokens (9 tiles), slot B up to 1024
    (8 tiles).  Gathered tokens ship transposed ([D, slots]) in fp8-e4m3,
    partition-major so every DMA row is one big contiguous descriptor.
  - Device (per core, identical SPMD program): dense SwiGLU FFN per slot,
    all matmuls in fp8 DoubleRow perf mode (K=256 per instruction):
        gT/uT = w1/w3.T @ xT   (PSUM fp32, [H_tile, tok])
        sT    = silu(gT/S1)                        (scalar engine, fp32)
        hT    = sT * uT                            (fp8, vector engine)
        y     = hT.T @ w2 -> y *= combine_weight   (bf16 out)
    Quantization scales: w1*64, w3*16, w2*64; x unscaled.  The net 1/1024
    is folded into the per-token combine weights on the host.
  - Host scatter-adds the (already weighted) expert outputs into the
    residual stream.

Hardcoded for B=2, S=4096, D=1024, H=512, E=16, K=2.
"""

import numpy as np

B, S, D, H, E, TOPK = 2, 4096, 1024, 512, 16, 2
T = B * S
NCORES = 8
EPC = E // NCORES          # experts per core = 2
SLOT_CAP = [1152, 1024]    # token capacity per slot (A, B)
SLOT_OFF = [0, 1152]
TOT = sum(SLOT_CAP)        # 2176 token slots per core
NTT = TOT // 128           # 17 token tiles per core
# (slot, token offset within slot, length) — one xt DMA slab per entry.
# n=512 blocks are optimal: DoubleRow at n=512 is moving-stream-bound
# (216ns/instr); measured n=384 drops to 259ns/instr (LDWEIGHTS-bound),
# so fewer, fatter blocks win even with a ragged 128 tail.  (Running the
# ragged block first was measured slower: the short cold-clock block plus
# DMA-wait gaps repeatedly reset the DVFS ramp.)
XBLOCKS = [(0, 0, 512), (0, 512, 512), (0, 1024, 128),
           (1, 0, 512), (1, 512, 512)]
ND = D // 128              # 8 d-tiles (stage-1 contraction)
NH = H // 128              # 4 h-tiles
NWB = 2 * NH * ND          # 64 weight blocks of 128 cols in w13
S1, S3, S2 = 64.0, 16.0, 64.0      # fp8 quantization scales
SY = 32.0                          # fp8 output scale (host divides back)
SCOMB = SY / (S3 * S2)             # folded into combine weights

_PROG = None
_LAST_RESULTS = None


def _build_program():
    import concourse.bacc as bacc
    import concourse.tile as tile
    from concourse import mybir

    BF = mybir.dt.bfloat16
    F8 = mybir.dt.float8e4
    F32 = mybir.dt.float32
    AF = mybir.ActivationFunctionType
    DR = mybir.MatmulPerfMode.DoubleRow

    nc = bacc.Bacc("TRN2", target_bir_lowering=False, debug=False,
                   num_devices=NCORES)

    # DRAM I/O (per core), all partition-major: row p holds everything
    # partition p will need, contiguously.
    xt_ds = [nc.dram_tensor(f"xtb{bi}", [128, ND * n], F8,
                            kind="ExternalInput")
             for bi, (_, _, n) in enumerate(XBLOCKS)]
    w13_d = nc.dram_tensor("w13", [EPC, 128, NWB * 128], F8,
                           kind="ExternalInput")
    w2_d = nc.dram_tensor("w2", [EPC, 128, NH * D], F8, kind="ExternalInput")
    wg_d = nc.dram_tensor("wg", [128, NTT], F32, kind="ExternalInput")
    y_d = nc.dram_tensor("y", [NTT, 128, D], F8, kind="ExternalOutput")

    w13 = w13_d.ap()
    w2 = w2_d.ap()
    wg = wg_d.ap()
    y = y_d.ap()

    with tile.TileContext(nc) as tc:
        with (
            tc.tile_pool(name="wpool", bufs=1) as wpool,
            tc.tile_pool(name="hpool", bufs=3) as hpool,
            tc.tile_pool(name="ypool", bufs=4) as ypool,
            tc.tile_pool(name="ps1", bufs=2, space="PSUM") as ps1,
            tc.tile_pool(name="ps2", bufs=4, space="PSUM") as ps2,
        ):
            # ---- HAM warmup: dummy matmuls on a zeroed scratch tile so the
            # PE clock ramp starts while the first input DMAs land (~2us).
            warm = wpool.tile([128, 512], BF, tag="warm", name="warm")
            nc.vector.memset(warm[:], 0.0)
            wps = ps1.tile([128, 512], F32, tag="g", name="wps")
            for i in range(13):
                nc.tensor.matmul(wps[:], warm[:, 0:128], warm[:],
                                 start=(i == 0), stop=(i == 12))

            # ---- static SBUF-resident inputs ----
            wg_sb = wpool.tile([128, NTT], F32, tag="wg", name="wg")
            xt_sb = [wpool.tile([128, ND, n], F8, tag=f"xtb{bi}",
                                name=f"xtb{bi}")
                     for bi, (_, _, n) in enumerate(XBLOCKS)]
            w13_sb = [wpool.tile([128, NWB, 128], F8, tag=f"w13_{e}",
                                 name=f"w13_{e}") for e in range(EPC)]
            w2_sb = [wpool.tile([128, NH, D], F8, tag=f"w2_{e}",
                                name=f"w2_{e}") for e in range(EPC)]

            # DMA issue order = need order, spread across idle engines.
            # critical first: w13[e0] ht0 (blocks 0..15) + xt block A0
            # (in two halves on separate queues so both land sooner)
            n0 = XBLOCKS[0][2]
            nc.sync.dma_start(w13_sb[0][:, 0:16, :], w13[0, :, 0:16 * 128])
            nc.scalar.dma_start(xt_sb[0][:, 0:4, :],
                                xt_ds[0].ap()[:, 0:4 * n0])
            nc.gpsimd.dma_start(xt_sb[0][:, 4:8, :],
                                xt_ds[0].ap()[:, 4 * n0:8 * n0])
            # weight slabs split at ht-consumption granularity: a reader
            # only waits on the chunk it needs, not the whole slab
            nc.sync.dma_start(w13_sb[0][:, 16:32, :],
                              w13[0, :, 16 * 128:32 * 128])
            nc.scalar.dma_start(xt_sb[1][:], xt_ds[1].ap()[:])
            nc.sync.dma_start(w13_sb[0][:, 32:48, :],
                              w13[0, :, 32 * 128:48 * 128])
            nc.sync.dma_start(w13_sb[0][:, 48:NWB, :],
                              w13[0, :, 48 * 128:NWB * 128])
            nc.sync.dma_start(w2_sb[0][:], w2[0])
            nc.scalar.dma_start(xt_sb[2][:], xt_ds[2].ap()[:])
            nc.sync.dma_start(w13_sb[1][:, 0:16, :], w13[1, :, 0:16 * 128])
            nc.scalar.dma_start(xt_sb[3][:], xt_ds[3].ap()[:])
            nc.sync.dma_start(w13_sb[1][:, 16:NWB, :],
                              w13[1, :, 16 * 128:NWB * 128])
            nc.sync.dma_start(w2_sb[1][:], w2[1])
            nc.scalar.dma_start(xt_sb[4][:], xt_ds[4].ap()[:])
            nc.gpsimd.dma_start(wg_sb[:], wg[:])

            # ---- compute: stage-1 of block b runs before stage-2 of
            # block b-1 so the scalar/vector h-pipeline has a full block
            # of slack before the PE needs h as stage-2 weights.
            def stage1(bi, e, n):
                h01 = [hpool.tile([128, 2, n], F8, tag=f"h{hp}_{n}",
                                  name=f"h{hp}_{bi}") for hp in range(2)]
                for ht in range(NH):
                    g = ps1.tile([128, 512], F32, tag="g", name="g")
                    u = ps1.tile([128, 512], F32, tag="u", name="u")
                    for w in range(2):
                        dst = g if w == 0 else u
                        for dp in range(ND // 2):
                            wb = (ht * 2 + w) * ND + 2 * dp
                            nc.tensor.matmul(
                                dst[:, :n],
                                w13_sb[e][:, wb:wb + 2, :],
                                xt_sb[bi][:, 2 * dp:2 * dp + 2, :],
                                start=(dp == 0), stop=(dp == ND // 2 - 1),
                                perf_mode=DR,
                            )
                    sg = hpool.tile([128, 512], F32, tag="sg", name="sg")
                    nc.scalar.activation(sg[:, :n], g[:, :n], AF.Silu,
                                         scale=1.0 / S1)
                    nc.vector.tensor_mul(h01[ht // 2][:, ht % 2, :],
                                         sg[:, :n], u[:, :n])
                return h01

            def stage2(e, off, n, h01):
                for tt in range(n // 128):
                    gtt = (SLOT_OFF[e] + off) // 128 + tt
                    ys = ypool.tile([128, D], F8, tag="ys", name="ys")
                    for db in range(2):
                        yp = ps2.tile([128, 512], F32, tag="yp", name="yp")
                        for hp in range(2):
                            nc.tensor.matmul(
                                yp[:],
                                h01[hp][:, :, tt * 128:(tt + 1) * 128],
                                w2_sb[e][:, 2 * hp:2 * hp + 2,
                                         db * 512:(db + 1) * 512],
                                start=(hp == 0), stop=(hp == 1),
                                perf_mode=DR,
                            )
                        # PSUM eviction split across vector+scalar halves
                        # so neither engine alone gates stage-2 drain
                        o = db * 512
                        nc.vector.tensor_scalar_mul(
                            ys[:, o:o + 256], yp[:, 0:256],
                            wg_sb[:, gtt:gtt + 1])
                        nc.scalar.mul(ys[:, o + 256:o + 512], yp[:, 256:512],
                                      wg_sb[:, gtt:gtt + 1])
                    nc.sync.dma_start(y[gtt], ys[:])

            prev = None
            for bi, (e, off, n) in enumerate(XBLOCKS):
                h01 = stage1(bi, e, n)
                if prev is not None:
                    stage2(*prev)
                prev = (e, off, n, h01)
            stage2(*prev)

    nc.compile()
    return nc


def _program():
    global _PROG
    if _PROG is None:
        _PROG = _build_program()
    return _PROG


def _route(x, gate_w):
    """fp32 softmax router + top-2 with renormalized weights (matches ref)."""
    logits = x @ gate_w.astype(np.float32)
    logits = logits - logits.max(axis=-1, keepdims=True)
    ex = np.exp(logits)
    scores = ex / ex.sum(axis=-1, keepdims=True)
    idx = np.argsort(-scores, axis=-1, kind="stable")[:, :TOPK]
    w = np.take_along_axis(scores, idx, axis=-1)
    w = w / w.sum(axis=-1, keepdims=True)
    return idx, w.astype(np.float32)


def _moe_numpy(x, gate_w, w1, w3, w2):
    """Slow exact fallback (only used if a capacity overflow ever happens)."""
    idx, wts = _route(x, gate_w)
    out = x.copy()
    for e in range(E):
        sel = np.nonzero(idx == e)
        toks = sel[0]
        ww = wts[sel]
        xe = x[toks]
        g = xe @ w1[e]
        u = xe @ w3[e]
        h = (g / (1.0 + np.exp(-g))) * u
        out[toks] += (h @ w2[e]) * ww[:, None]
    return out


def _pack_w13(a):
    """[D, 2H] -> [128, NWB*128], columns ordered (ht, w1|w3, dt, 128)."""
    r = a.reshape(ND, 128, 2, NH, 128)        # dt, p, w, ht, c
    r = r.transpose(1, 3, 2, 0, 4)            # p, ht, w, dt, c
    return np.ascontiguousarray(r.reshape(128, NWB * 128))


def _pmajor(a, cols):
    """[rows=nd*128, cols] -> [128, nd*cols] partition-major layout."""
    nd = a.shape[0] // 128
    return np.ascontiguousarray(
        a.reshape(nd, 128, cols).transpose(1, 0, 2).reshape(128, nd * cols))


def kernel(hidden_states, gate_w, w1, w3, w2):
    import ml_dtypes
    from concourse import bass_utils

    F8NP = ml_dtypes.float8_e4m3

    hidden_states = np.asarray(hidden_states, dtype=np.float32)
    gate_w = np.asarray(gate_w, dtype=np.float32)
    w1 = np.asarray(w1, dtype=np.float32)
    w3 = np.asarray(w3, dtype=np.float32)
    w2 = np.asarray(w2, dtype=np.float32)

    x = hidden_states.reshape(T, D)
    idx, wts = _route(x, gate_w)

    tok_lists = []
    wt_lists = []
    for e in range(E):
        sel = np.nonzero(idx == e)
        tok_lists.append(sel[0])
        wt_lists.append(wts[sel])
    counts = np.array([len(t) for t in tok_lists])

    # pair largest with smallest; slot A = larger of the pair
    order = np.argsort(-counts, kind="stable")
    pairs = [(order[i], order[E - 1 - i]) for i in range(NCORES)]
    if any(counts[a] > SLOT_CAP[0] or counts[b] > SLOT_CAP[1]
           for a, b in pairs):
        return _moe_numpy(x, gate_w, w1, w3, w2).reshape(B, S, D)

    xq = x.astype(F8NP)
    w13q = np.concatenate([w1 * S1, w3 * S3], axis=2).astype(F8NP)
    w13q = np.stack([_pack_w13(w13q[e]) for e in range(E)])
    w2q = (w2 * S2).astype(F8NP)
    w2q = np.stack([_pmajor(w2q[e], D) for e in range(E)])

    in_maps = []
    for c in range(NCORES):
        xg = np.zeros((TOT, D), dtype=F8NP)
        wgt = np.zeros(TOT, dtype=np.float32)
        for j, e in enumerate(pairs[c]):
            ne = counts[e]
            xg[SLOT_OFF[j]:SLOT_OFF[j] + ne] = xq[tok_lists[e]]
            wgt[SLOT_OFF[j]:SLOT_OFF[j] + ne] = wt_lists[e] * SCOMB
        xgT = np.ascontiguousarray(xg.T)       # [D, TOT]
        ea, eb = pairs[c]
        m = {
            "w13": np.stack([w13q[ea], w13q[eb]]),
            "w2": np.stack([w2q[ea], w2q[eb]]),
            "wg": np.ascontiguousarray(wgt.reshape(NTT, 128).T),
        }
        for bi, (s, off, n) in enumerate(XBLOCKS):
            c0 = SLOT_OFF[s] + off
            m[f"xtb{bi}"] = _pmajor(xgT[:, c0:c0 + n], n)
        in_maps.append(m)

    res = bass_utils.run_bass_kernel_spmd(
        _program(), in_maps, core_ids=list(range(NCORES)))
    global _LAST_RESULTS
    _LAST_RESULTS = res

    out = x.copy()
    for c in range(NCORES):
        yc = np.asarray(res.results[c]["y"], dtype=np.float32) * (1.0 / SY)
        yc = yc.reshape(NTT * 128, D)
        for j, e in enumerate(pairs[c]):
            ne = counts[e]
            out[tok_lists[e]] += yc[SLOT_OFF[j]:SLOT_OFF[j] + ne]
    return out.reshape(B, S, D)


# revision 32
# speedup vs baseline: 1.0411x; 1.0411x over previous
"""Trainium2 Bass kernel: DeepSeek-style MoE layer (16 experts, top-2).

Strategy (expert-parallel, 8 cores):
  - Host computes the router (softmax + top-2 + renorm) in fp32 numpy and
    builds the token dispatch.  Experts are paired large-with-small onto
    cores; slot A holds up to 1152 tokens (9 tiles), slot B up to 1024
    (8 tiles).  Gathered tokens ship transposed ([D, slots]) in fp8-e4m3,
    partition-major so every DMA row is one big contiguous descriptor.
  - Device (per core, identical SPMD program): dense SwiGLU FFN per slot,
    all matmuls in fp8 DoubleRow perf mode (K=256 per instruction):
        gT/uT = w1/w3.T @ xT   (PSUM fp32, [H_tile, tok])
        sT    = silu(gT/S1)                        (scalar engine, fp32)
        hT    = sT * uT                            (fp8, vector engine)
        y     = hT.T @ w2 -> y *= combine_weight   (bf16 out)
    Quantization scales: w1*64, w3*16, w2*64; x unscaled.  The net 1/1024
    is folded into the per-token combine weights on the host.
  - Host scatter-adds the (already weighted) expert outputs into the
    residual stream.

Hardcoded for B=2, S=4096, D=1024, H=512, E=16, K=2.
"""

import numpy as np

B, S, D, H, E, TOPK = 2, 4096, 1024, 512, 16, 2
T = B * S
NCORES = 8
EPC = E // NCORES          # experts per core = 2
SLOT_CAP = [1152, 1024]    # token capacity per slot (A, B)
SLOT_OFF = [0, 1152]
TOT = sum(SLOT_CAP)        # 2176 token slots per core
NTT = TOT // 128           # 17 token tiles per core
# (slot, token offset within slot, length) — one xt DMA slab per entry.
# n=512 blocks are optimal: DoubleRow at n=512 is moving-stream-bound
# (216ns/instr); measured n=384 drops to 259ns/instr (LDWEIGHTS-bound),
# so fewer, fatter blocks win even with a ragged 128 tail.  (Running the
# ragged block first was measured slower: the short cold-clock block plus
# DMA-wait gaps repeatedly reset the DVFS ramp.)
XBLOCKS = [(0, 0, 512), (0, 512, 512), (0, 1024, 128),
           (1, 0, 512), (1, 512, 512)]
ND = D // 128              # 8 d-tiles (stage-1 contraction)
NH = H // 128              # 4 h-tiles
NWB = 2 * NH * ND          # 64 weight blocks of 128 cols in w13
S1, S3, S2 = 64.0, 16.0, 64.0      # fp8 quantization scales
SY = 32.0                          # fp8 output scale (host divides back)
SCOMB = SY / (S3 * S2)             # folded into combine weights

_PROG = None
_LAST_RESULTS = None


def _build_program():
    import concourse.bacc as bacc
    import concourse.tile as tile
    from concourse import mybir

    BF = mybir.dt.bfloat16
    F8 = mybir.dt.float8e4
    F32 = mybir.dt.float32
    AF = mybir.ActivationFunctionType
    DR = mybir.MatmulPerfMode.DoubleRow

    nc = bacc.Bacc("TRN2", target_bir_lowering=False, debug=False,
                   num_devices=NCORES)

    # DRAM I/O (per core), all partition-major: row p holds everything
    # partition p will need, contiguously.
    xt_ds = [nc.dram_tensor(f"xtb{bi}", [128, ND * n], F8,
                            kind="ExternalInput")
             for bi, (_, _, n) in enumerate(XBLOCKS)]
    w13_d = nc.dram_tensor("w13", [EPC, 128, NWB * 128], F8,
                           kind="ExternalInput")
    w2_d = nc.dram_tensor("w2", [EPC, 128, NH * D], F8, kind="ExternalInput")
    wg_d = nc.dram_tensor("wg", [128, NTT], F32, kind="ExternalInput")
    y_d = nc.dram_tensor("y", [NTT, 128, D], F8, kind="ExternalOutput")

    w13 = w13_d.ap()
    w2 = w2_d.ap()
    wg = wg_d.ap()
    y = y_d.ap()

    with tile.TileContext(nc) as tc:
        with (
            tc.tile_pool(name="wpool", bufs=1) as wpool,
            tc.tile_pool(name="hpool", bufs=2) as hpool,
            tc.tile_pool(name="ypool", bufs=4) as ypool,
            tc.tile_pool(name="ps1", bufs=2, space="PSUM") as ps1,
            tc.tile_pool(name="ps2", bufs=4, space="PSUM") as ps2,
        ):
            # ---- HAM warmup: dummy matmuls on a zeroed scratch tile so the
            # PE clock ramp starts while the first input DMAs land (~2us).
            warm = wpool.tile([128, 512], BF, tag="warm", name="warm")
            nc.vector.memset(warm[:], 0.0)
            wps = ps1.tile([128, 512], F32, tag="g", name="wps")
            for i in range(13):
                nc.tensor.matmul(wps[:], warm[:, 0:128], warm[:],
                                 start=(i == 0), stop=(i == 12))

            # ---- static SBUF-resident inputs ----
            wg_sb = wpool.tile([128, NTT], F32, tag="wg", name="wg")
            xt_sb = [wpool.tile([128, ND, n], F8, tag=f"xtb{bi}",
                                name=f"xtb{bi}")
                     for bi, (_, _, n) in enumerate(XBLOCKS)]
            w13_sb = [wpool.tile([128, NWB, 128], F8, tag=f"w13_{e}",
                                 name=f"w13_{e}") for e in range(EPC)]
            w2_sb = [wpool.tile([128, NH, D], F8, tag=f"w2_{e}",
                                name=f"w2_{e}") for e in range(EPC)]

            # DMA issue order = need order, spread across idle engines.
            # critical first: w13[e0] ht0 (blocks 0..15) + xt block A0
            # (in two halves on separate queues so both land sooner)
            n0 = XBLOCKS[0][2]
            nc.sync.dma_start(w13_sb[0][:, 0:16, :], w13[0, :, 0:16 * 128])
            nc.scalar.dma_start(xt_sb[0][:, 0:4, :],
                                xt_ds[0].ap()[:, 0:4 * n0])
            nc.gpsimd.dma_start(xt_sb[0][:, 4:8, :],
                                xt_ds[0].ap()[:, 4 * n0:8 * n0])
            # ht1 chunk separate so stage-1 ht1 doesn't wait on the whole
            # remaining slab's completion semaphore (measured 2us stall)
            nc.sync.dma_start(w13_sb[0][:, 16:32, :],
                              w13[0, :, 16 * 128:32 * 128])
            nc.sync.dma_start(w13_sb[0][:, 32:NWB, :],
                              w13[0, :, 32 * 128:NWB * 128])
            nc.scalar.dma_start(xt_sb[1][:], xt_ds[1].ap()[:])
            nc.sync.dma_start(w2_sb[0][:], w2[0])
            nc.scalar.dma_start(xt_sb[2][:], xt_ds[2].ap()[:])
            nc.sync.dma_start(w13_sb[1][:], w13[1])
            nc.scalar.dma_start(xt_sb[3][:], xt_ds[3].ap()[:])
            nc.sync.dma_start(w2_sb[1][:], w2[1])
            nc.scalar.dma_start(xt_sb[4][:], xt_ds[4].ap()[:])
            nc.gpsimd.dma_start(wg_sb[:], wg[:])

            # ---- compute: stage-1 of block b runs before stage-2 of
            # block b-1 so the scalar/vector h-pipeline has a full block
            # of slack before the PE needs h as stage-2 weights.
            def stage1(bi, e, n):
                h01 = [hpool.tile([128, 2, n], F8, tag=f"h{hp}_{n}",
                                  name=f"h{hp}_{bi}") for hp in range(2)]
                for ht in range(NH):
                    g = ps1.tile([128, 512], F32, tag="g", name="g")
                    u = ps1.tile([128, 512], F32, tag="u", name="u")
                    for w in range(2):
                        dst = g if w == 0 else u
                        for dp in range(ND // 2):
                            wb = (ht * 2 + w) * ND + 2 * dp
                            nc.tensor.matmul(
                                dst[:, :n],
                                w13_sb[e][:, wb:wb + 2, :],
                                xt_sb[bi][:, 2 * dp:2 * dp + 2, :],
                                start=(dp == 0), stop=(dp == ND // 2 - 1),
                                perf_mode=DR,
                            )
                    sg = hpool.tile([128, 512], F32, tag="sg", name="sg")
                    nc.scalar.activation(sg[:, :n], g[:, :n], AF.Silu,
                                         scale=1.0 / S1)
                    nc.vector.tensor_mul(h01[ht // 2][:, ht % 2, :],
                                         sg[:, :n], u[:, :n])
                return h01

            def stage2(e, off, n, h01):
                for tt in range(n // 128):
                    gtt = (SLOT_OFF[e] + off) // 128 + tt
                    ys = ypool.tile([128, D], F8, tag="ys", name="ys")
                    for db in range(2):
                        yp = ps2.tile([128, 512], F32, tag="yp", name="yp")
                        for hp in range(2):
                            nc.tensor.matmul(
                                yp[:],
                                h01[hp][:, :, tt * 128:(tt + 1) * 128],
                                w2_sb[e][:, 2 * hp:2 * hp + 2,
                                         db * 512:(db + 1) * 512],
                                start=(hp == 0), stop=(hp == 1),
                                perf_mode=DR,
                            )
                        # PSUM eviction split across vector+scalar halves
                        # so neither engine alone gates stage-2 drain
                        o = db * 512
                        nc.vector.tensor_scalar_mul(
                            ys[:, o:o + 256], yp[:, 0:256],
                            wg_sb[:, gtt:gtt + 1])
                        nc.scalar.mul(ys[:, o + 256:o + 512], yp[:, 256:512],
                                      wg_sb[:, gtt:gtt + 1])
                    nc.sync.dma_start(y[gtt], ys[:])

            prev = None
            for bi, (e, off, n) in enumerate(XBLOCKS):
                h01 = stage1(bi, e, n)
                if prev is not None:
                    stage2(*prev)
                prev = (e, off, n, h01)
            stage2(*prev)

    nc.compile()
    return nc


def _program():
    global _PROG
    if _PROG is None:
        _PROG = _build_program()
    return _PROG


def _route(x, gate_w):
    """fp32 softmax router + top-2 with renormalized weights (matches ref)."""
    logits = x @ gate_w.astype(np.float32)
    logits = logits - logits.max(axis=-1, keepdims=True)
    ex = np.exp(logits)
    scores = ex / ex.sum(axis=-1, keepdims=True)
    idx = np.argsort(-scores, axis=-1, kind="stable")[:, :TOPK]
    w = np.take_along_axis(scores, idx, axis=-1)
    w = w / w.sum(axis=-1, keepdims=True)
    return idx, w.astype(np.float32)


def _moe_numpy(x, gate_w, w1, w3, w2):
    """Slow exact fallback (only used if a capacity overflow ever happens)."""
    idx, wts = _route(x, gate_w)
    out = x.copy()
    for e in range(E):
        sel = np.nonzero(idx == e)
        toks = sel[0]
        ww = wts[sel]
        xe = x[toks]
        g = xe @ w1[e]
        u = xe @ w3[e]
        h = (g / (1.0 + np.exp(-g))) * u
        out[toks] += (h @ w2[e]) * ww[:, None]
    return out


def _pack_w13(a):
    """[D, 2H] -> [128, NWB*128], columns ordered (ht, w1|w3, dt, 128)."""
    r = a.reshape(ND, 128, 2, NH, 128)        # dt, p, w, ht, c
    r = r.transpose(1, 3, 2, 0, 4)            # p, ht, w, dt, c
    return np.ascontiguousarray(r.reshape(128, NWB * 128))


def _pmajor(a, cols):
    """[rows=nd*128, cols] -> [128, nd*cols] partition-major layout."""
    nd = a.shape[0] // 128
    return np.ascontiguousarray(
        a.reshape(nd, 128, cols).transpose(1, 0, 2).reshape(128, nd * cols))


def kernel(hidden_states, gate_w, w1, w3, w2):
    import ml_dtypes
    from concourse import bass_utils

    F8NP = ml_dtypes.float8_e4m3

    hidden_states = np.asarray(hidden_states, dtype=np.float32)
    gate_w = np.asarray(gate_w, dtype=np.float32)
    w1 = np.asarray(w1, dtype=np.float32)
    w3 = np.asarray(w3, dtype=np.float32)
    w2 = np.asarray(w2, dtype=np.float32)

    x = hidden_states.reshape(T, D)
    idx, wts = _route(x, gate_w)

    tok_lists = []
    wt_lists = []
    for e in range(E):
        sel = np.nonzero(idx == e)
        tok_lists.append(sel[0])
        wt_lists.append(wts[sel])
    counts = np.array([len(t) for t in tok_lists])

    # pair largest with smallest; slot A = larger of the pair
    order = np.argsort(-counts, kind="stable")
    pairs = [(order[i], order[E - 1 - i]) for i in range(NCORES)]
    if any(counts[a] > SLOT_CAP[0] or counts[b] > SLOT_CAP[1]
           for a, b in pairs):
        return _moe_numpy(x, gate_w, w1, w3, w2).reshape(B, S, D)

    xq = x.astype(F8NP)
    w13q = np.concatenate([w1 * S1, w3 * S3], axis=2).astype(F8NP)
    w13q = np.stack([_pack_w13(w13q[e]) for e in range(E)])
    w2q = (w2 * S2).astype(F8NP)
    w2q = np.stack([_pmajor(w2q[e], D) for e in range(E)])

    in_maps = []
    for c in range(NCORES):
        xg = np.zeros((TOT, D), dtype=F8NP)
        wgt = np.zeros(TOT, dtype=np.float32)
        for j, e in enumerate(pairs[c]):
            ne = counts[e]
            xg[SLOT_OFF[j]:SLOT_OFF[j] + ne] = xq[tok_lists[e]]
            wgt[SLOT_OFF[j]:SLOT_OFF[j] + ne] = wt_lists[e] * SCOMB
        xgT = np.ascontiguousarray(xg.T)       # [D, TOT]
        ea, eb = pairs[c]
        m = {
            "w13": np.stack([w13q[ea], w13q[eb]]),
            "w2": np.stack([w2q[ea], w2q[eb]]),
            "wg": np.ascontiguousarray(wgt.reshape(NTT, 128).T),
        }
        for bi, (s, off, n) in enumerate(XBLOCKS):
            c0 = SLOT_OFF[s] + off
            m[f"xtb{bi}"] = _pmajor(xgT[:, c0:c0 + n], n)
        in_maps.append(m)

    res = bass_utils.run_bass_kernel_spmd(
        _program(), in_maps, core_ids=list(range(NCORES)))
    global _LAST_RESULTS
    _LAST_RESULTS = res

    out = x.copy()
    for c in range(NCORES):
        yc = np.asarray(res.results[c]["y"], dtype=np.float32) * (1.0 / SY)
        yc = yc.reshape(NTT * 128, D)
        for j, e in enumerate(pairs[c]):
            ne = counts[e]
            out[tok_lists[e]] += yc[SLOT_OFF[j]:SLOT_OFF[j] + ne]
    return out.reshape(B, S, D)
